# revision 1
# baseline (speedup 1.0000x reference)
"""Bass/Tile kernel for one dense transformer block (B=128,T=256,E=512,H=8,F=2048),
data-parallel over batch across 8 NeuronCores (16 batches/core).

Per-core plan (4096 tokens = 16 chunks of 1 batch / 256 tokens):
  LN1 (bn_stats) in [t,e] -> PE-transpose xhat (bf16) -> h1T [e,t]; ln g/b
  applied after the transpose on DVE (per-partition there) with free-dim
  broadcast APs.
  QKV (bf16 matmuls, fp32 psum): Q^T,K^T = W^T @ h1T; V in [t,hd] (+ones col).
  Attention per head: S^T = K^T.T @ Q^T ([s,t], K=64); E^T = exp(S^T/8) on ACT
  (bf16 out); causal mask via 0/1 multiplies; PV: [V|1].T @ E^T -> ctx_u^T
  (rows 0:64) + denominators (row 64); ctx_u and denominator rows copied to
  SBUF; ONE batched reciprocal [8,256] per chunk; per head-pair the recip row
  is partition-broadcast via a K=8 selector matmul; normalize into ctxnT.
  Wo (bf16) + bias (K=1 ones matmul) + residual -> x2 (fp32).
  LN2 -> h2T; FFN1 (bf16): a^T = relu(W1^T h2 + b1) [f,t] (b1 per-partition in
  ACT relu); FFN2 (bf16) + b2 + residual -> y (fp32).
Matmul operands are bf16 (1 cycle/column on PE); accumulation is fp32 in PSUM;
LN statistics, softmax denominators, residuals stay fp32/fp32r."""

import numpy as np
from contextlib import ExitStack

import ml_dtypes
import concourse.bass as bass
import concourse.mybir as mybir
import concourse.tile as tile
from concourse import bacc
from concourse.bass import ts, ds

AF = mybir.ActivationFunctionType
ALU = mybir.AluOpType
FP32 = mybir.dt.float32
FP32R = mybir.dt.float32r
BF16 = mybir.dt.bfloat16

B, T, E, H = 128, 256, 512, 8
D = E // H          # 64
F = 4 * E           # 2048
NCORES = 8
BS = B // NCORES    # 16 batches per core
P = 128
EPS = 1e-5
NTOK = BS * T       # 4096 tokens per core


def build(n_chunks: int = BS):
    nc = bacc.Bacc("TRN2", target_bir_lowering=False, debug=False)

    x_d = nc.dram_tensor("x", [NTOK, E], FP32, kind="ExternalInput").ap()
    wq_d = nc.dram_tensor("wq", [E, E], BF16, kind="ExternalInput").ap()
    wk_d = nc.dram_tensor("wk", [E, E], BF16, kind="ExternalInput").ap()
    wv_d = nc.dram_tensor("wv", [E, E], BF16, kind="ExternalInput").ap()
    wo_d = nc.dram_tensor("wo", [E, E], BF16, kind="ExternalInput").ap()
    bo_d = nc.dram_tensor("bo", [E], BF16, kind="ExternalInput").ap()
    g1_d = nc.dram_tensor("g1", [E], FP32, kind="ExternalInput").ap()
    be1_d = nc.dram_tensor("be1", [E], FP32, kind="ExternalInput").ap()
    g2_d = nc.dram_tensor("g2", [E], FP32, kind="ExternalInput").ap()
    be2_d = nc.dram_tensor("be2", [E], FP32, kind="ExternalInput").ap()
    w1_d = nc.dram_tensor("w1", [E, F], BF16, kind="ExternalInput").ap()
    b1_d = nc.dram_tensor("b1", [F], FP32, kind="ExternalInput").ap()
    w2_d = nc.dram_tensor("w2", [F, E], BF16, kind="ExternalInput").ap()
    b2_d = nc.dram_tensor("b2", [E], BF16, kind="ExternalInput").ap()
    triu_d = nc.dram_tensor("triu", [P, P], BF16, kind="ExternalInput").ap()
    mask2_d = nc.dram_tensor("mask2", [P, T], BF16, kind="ExternalInput").ap()
    ident_d = nc.dram_tensor("ident", [P, P], BF16, kind="ExternalInput").ap()
    ones_d = nc.dram_tensor("ones", [1, P], BF16, kind="ExternalInput").ap()
    y_d = nc.dram_tensor("y", [NTOK, E], FP32, kind="ExternalOutput").ap()

    with tile.TileContext(nc) as tc, ExitStack() as ctx:
        # ---------------- persistent weights ----------------
        wpool = ctx.enter_context(tc.tile_pool(name="weights", bufs=1))
        wq_sb = wpool.tile([P, 4, E], BF16, name="wq_sb", tag="wq_sb")
        wk_sb = wpool.tile([P, 4, E], BF16, name="wk_sb", tag="wk_sb")
        wv_sb = wpool.tile([P, 4, E], BF16, name="wv_sb", tag="wv_sb")
        wo_sb = wpool.tile([P, 4, E], BF16, name="wo_sb", tag="wo_sb")
        w1_sb = wpool.tile([P, 4, F], BF16, name="w1_sb", tag="w1_sb")
        w2_sb = wpool.tile([P, 16, E], BF16, name="w2_sb", tag="w2_sb")
        b1_sb = wpool.tile([P, 16], FP32, name="b1_sb", tag="b1_sb")
        bo_sb = wpool.tile([1, E], BF16, name="bo_sb", tag="bo_sb")
        b2_sb = wpool.tile([1, E], BF16, name="b2_sb", tag="b2_sb")
        g1_sb = wpool.tile([P, 4], FP32, name="g1_sb", tag="g1_sb")
        be1_sb = wpool.tile([P, 4], FP32, name="be1_sb", tag="be1_sb")
        g2_sb = wpool.tile([P, 4], FP32, name="g2_sb", tag="g2_sb")
        be2_sb = wpool.tile([P, 4], FP32, name="be2_sb", tag="be2_sb")
        triu_sb = wpool.tile([P, P], BF16, name="triu_sb", tag="triu_sb")
        mask2_sb = wpool.tile([P, T], BF16, name="mask2_sb", tag="mask2_sb")
        ident_sb = wpool.tile([P, P], BF16, name="ident_sb", tag="ident_sb")
        ones_sb = wpool.tile([1, P], BF16, name="ones_sb", tag="ones_sb")
        eps_sb = wpool.tile([P, 1], FP32, name="eps_sb", tag="eps_sb")
        nc.gpsimd.memset(eps_sb, EPS)
        onescol_sb = wpool.tile([P, H], FP32, name="onescol_sb", tag="onescol_sb")
        nc.gpsimd.memset(onescol_sb, 1.0)

        nc.sync.dma_start(wq_sb, wq_d.rearrange("(eo ei) f -> ei eo f", ei=P))
        nc.sync.dma_start(wk_sb, wk_d.rearrange("(eo ei) f -> ei eo f", ei=P))
        nc.sync.dma_start(wv_sb, wv_d.rearrange("(eo ei) f -> ei eo f", ei=P))
        nc.sync.dma_start(wo_sb, wo_d.rearrange("(eo ei) f -> ei eo f", ei=P))
        nc.sync.dma_start(w1_sb, w1_d.rearrange("(eo ei) f -> ei eo f", ei=P))
        nc.sync.dma_start(w2_sb, w2_d.rearrange("(fo fi) e -> fi fo e", fi=P))
        nc.sync.dma_start(b1_sb, b1_d.rearrange("(fo fi) -> fi fo", fi=P))
        nc.sync.dma_start(bo_sb, bo_d[None, :])
        nc.sync.dma_start(b2_sb, b2_d[None, :])
        nc.sync.dma_start(g1_sb, g1_d.rearrange("(eo ei) -> ei eo", ei=P))
        nc.sync.dma_start(be1_sb, be1_d.rearrange("(eo ei) -> ei eo", ei=P))
        nc.sync.dma_start(g2_sb, g2_d.rearrange("(eo ei) -> ei eo", ei=P))
        nc.sync.dma_start(be2_sb, be2_d.rearrange("(eo ei) -> ei eo", ei=P))
        nc.sync.dma_start(triu_sb, triu_d)
        nc.sync.dma_start(mask2_sb, mask2_d)
        nc.sync.dma_start(ident_sb, ident_d)
        nc.sync.dma_start(ones_sb, ones_d)

        # ---------------- working pools ----------------
        sb = ctx.enter_context(tc.tile_pool(name="work", bufs=2))
        ps = ctx.enter_context(tc.tile_pool(name="psum", bufs=1, space="PSUM"))

        # one-time bias broadcast tiles (bias add then rides GPSIMD, off PE)
        bo_bc = wpool.tile([P, E], FP32, name="bo_bc", tag="bo_bc")
        b2_bc = wpool.tile([P, E], FP32, name="b2_bc", tag="b2_bc")
        for bias_sb, bias_bc, bnm in ((bo_sb, bo_bc, "bo"), (b2_sb, b2_bc, "b2")):
            ps_bias = ps.tile([P, E], FP32, name=f"ps_bias_{bnm}", tag="big", bufs=3)
            nc.tensor.matmul(ps_bias, ones_sb, bias_sb, start=True, stop=True)
            nc.vector.tensor_copy(bias_bc, ps_bias)

        def layer_norm_T(x_ap, g_ap, b_ap, hT, j):
            """LN over free dim of x_ap [128t, 512e]; write bf16 transposed
            (g/b applied per-partition post-transpose) into hT[:, :, ts(j,128)]."""
            stats = sb.tile([P, 6], FP32, name="stats", tag="stats", bufs=2)
            nc.vector.bn_stats(stats, x_ap)
            mv = sb.tile([P, 2], FP32, name="mv", tag="mv", bufs=2)
            nc.vector.bn_aggr(mv, stats)
            rstd = sb.tile([P, 1], FP32, name="rstd", tag="rstd", bufs=2)
            nc.scalar.activation(rstd, mv[:, 1:2], AF.Sqrt, bias=eps_sb)
            nc.vector.reciprocal(rstd, rstd)
            xh = sb.tile([P, E], BF16, name="xh", tag="xh", bufs=2)
            nc.vector.tensor_scalar(
                out=xh, in0=x_ap, scalar1=mv[:, 0:1], scalar2=rstd,
                op0=ALU.subtract, op1=ALU.mult)
            ps_tr = ps.tile([P, 4, P], BF16, name="ps_tr", tag="big", bufs=3)
            for eo in range(4):
                nc.tensor.transpose(ps_tr[:, eo, :], xh[:, ts(eo, P)], ident_sb)
            hslice = hT[:, :, ts(j, P)]
            nc.vector.tensor_mul(hslice, ps_tr,
                                 g_ap[:, :, None].to_broadcast([P, 4, P]))
            nc.vector.tensor_add(hslice, hslice,
                                 b_ap[:, :, None].to_broadcast([P, 4, P]))

        CT = 2 * T  # 512-token chunk = 2 batches
        for c in range(n_chunks // 2):
            rows = ds(c * CT, CT)
            x_t = sb.tile([P, 4, E], FP32, name="x_t", tag="x_t")
            nc.sync.dma_start(x_t, x_d[rows, :].rearrange("(j p) e -> p j e", p=P))

            # ---- LN1 + transpose ----
            h1T = sb.tile([P, 4, CT], BF16, name="h1T", tag="h1T", bufs=3)
            for j in range(4):
                layer_norm_T(x_t[:, j, :], g1_sb, be1_sb, h1T, j)

            # ---- Q^T, K^T projections: [hd, t] (N=512 moving) ----
            qT = sb.tile([P, 4, CT], BF16, name="qT", tag="qT", bufs=3)
            kT = sb.tile([P, 4, CT], BF16, name="kT", tag="kT", bufs=3)
            for w_sb, outT, nm in ((wq_sb, qT, "q"), (wk_sb, kT, "k")):
                for m in range(4):
                    ps_p = ps.tile([P, CT], FP32, name=f"ps_{nm}", tag="big", bufs=3)
                    for ke in range(4):
                        nc.tensor.matmul(
                            ps_p, w_sb[:, ke, ts(m, P)],
                            h1T[:, ke, :], start=(ke == 0), stop=(ke == 3))
                    nc.vector.tensor_copy(outT[:, m, :], ps_p)

            # ---- V projection: [t, hd] (+ones col) ----
            v_sb = sb.tile([P, 4, H, D + 1], BF16, name="v_sb", tag="v_sb", bufs=3)
            for j in range(4):
                ps_v = ps.tile([P, E], FP32, name="ps_v", tag="big", bufs=3)
                for ke in range(4):
                    nc.tensor.matmul(
                        ps_v, h1T[:, ke, ts(j, P)], wv_sb[:, ke],
                        start=(ke == 0), stop=(ke == 3))
                nc.vector.tensor_copy(
                    v_sb[:, j, :, 0:D], ps_v.rearrange("p (h d) -> p h d", h=H))
                nc.vector.tensor_copy(v_sb[:, j, :, D:D + 1], onescol_sb[:, :, None])

            # ---- attention per (batch, head) ----
            ctxnT = sb.tile([P, 4, CT], BF16, name="ctxnT", tag="ctxnT")
            for b in range(2):
                t0 = b * T
                for h in range(H):
                    p0 = (h % 2) * 64
                    hdo = h // 2
                    ps_st = ps.tile([P, 2, T], FP32, name="ps_st", tag="st", bufs=1)
                    nc.tensor.matmul(
                        ps_st[:, 0, :], kT[p0:p0 + 64, hdo, ds(t0, P)],
                        qT[p0:p0 + 64, hdo, ds(t0, T)], start=True, stop=True)
                    nc.tensor.matmul(
                        ps_st[:, 1, P:T], kT[p0:p0 + 64, hdo, ds(t0 + P, P)],
                        qT[p0:p0 + 64, hdo, ds(t0 + P, P)], start=True, stop=True)
                    eT = sb.tile([P, 2, T], BF16, name="eT", tag="eT", bufs=3)
                    nc.scalar.activation(eT[:, 0, :], ps_st[:, 0, :], AF.Exp,
                                         scale=float(D) ** -0.5)
                    nc.scalar.activation(eT[:, 1, P:T], ps_st[:, 1, P:T], AF.Exp,
                                         scale=float(D) ** -0.5)
                    nc.vector.tensor_mul(eT[:, 0, 0:P], eT[:, 0, 0:P], triu_sb)
                    nc.vector.tensor_mul(eT[:, 1, P:T], eT[:, 1, P:T], triu_sb)
                    ps_pv = ps.tile([D + 1, T], FP32, name="ps_pv", tag="pv", bufs=2)
                    nc.tensor.matmul(ps_pv, v_sb[:, 2 * b, h, :],
                                     eT[:, 0, :], start=True, stop=False)
                    nc.tensor.matmul(ps_pv[:, P:T], v_sb[:, 2 * b + 1, h, :],
                                     eT[:, 1, P:T], start=False, stop=True)
                    ctxu = sb.tile([64, T], BF16, name="ctxu", tag="ctxu", bufs=2)
                    nc.vector.tensor_copy(ctxu, ps_pv[0:D, :])
                    l_row = sb.tile([1, T], FP32, name="l_row", tag="l_row", bufs=2)
                    nc.vector.tensor_copy(l_row, ps_pv[D:D + 1, :])
                    rec = sb.tile([1, T], FP32, name="rec", tag="rec", bufs=2)
                    nc.vector.reciprocal_approx_fast(rec, l_row)
                    rec_bf = sb.tile([1, T], BF16, name="rec_bf", tag="rec_bf", bufs=2)
                    nc.vector.tensor_copy(rec_bf, rec)
                    ps_bc = ps.tile([64, T], FP32, name="ps_bc", tag="bc", bufs=1)
                    nc.tensor.matmul(ps_bc, ones_sb[0:1, 0:64], rec_bf,
                                     start=True, stop=True)
                    nc.vector.tensor_mul(ctxnT[p0:p0 + 64, hdo, ds(t0, T)],
                                         ctxu, ps_bc)

            # ---- Wo projection + bias + residual -> x2 ----
            x2_t = sb.tile([P, 4, E], FP32, name="x2_t", tag="x2_t")
            for tb in range(4):
                ps_o = ps.tile([P, E], FP32, name="ps_o", tag="big", bufs=3)
                for hdo in range(4):
                    nc.tensor.matmul(ps_o, ctxnT[:, hdo, ts(tb, P)],
                                     wo_sb[:, hdo, :],
                                     start=(hdo == 0), stop=(hdo == 3))
                nc.vector.tensor_add(x2_t[:, tb, :], ps_o, x_t[:, tb, :])
                nc.gpsimd.tensor_add(x2_t[:, tb, :], x2_t[:, tb, :], bo_bc)

            # ---- LN2 + transpose ----
            h2T = sb.tile([P, 4, CT], BF16, name="h2T", tag="h2T", bufs=3)
            for j in range(4):
                layer_norm_T(x2_t[:, j, :], g2_sb, be2_sb, h2T, j)

            # ---- FFN1: a^T = relu(W1^T h2 + b1) in [f, t] (N=512) ----
            aT = sb.tile([P, 16, CT], BF16, name="aT", tag="aT", bufs=2)
            for fb in range(16):
                ps_f1 = ps.tile([P, CT], FP32, name="ps_f1", tag="big", bufs=3)
                for ke in range(4):
                    nc.tensor.matmul(ps_f1, w1_sb[:, ke, ts(fb, P)],
                                     h2T[:, ke, :],
                                     start=(ke == 0), stop=(ke == 3))
                nc.scalar.activation(aT[:, fb, :], ps_f1, AF.Relu,
                                     bias=b1_sb[:, fb:fb + 1])

            # ---- FFN2 + bias + residual -> y ----
            for tb in range(4):
                ps_f2 = ps.tile([P, E], FP32, name="ps_f2", tag="f2", bufs=1)
                for fo in range(16):
                    nc.tensor.matmul(ps_f2, aT[:, fo, ts(tb, P)],
                                     w2_sb[:, fo, :],
                                     start=(fo == 0), stop=(fo == 15))
                y_t = sb.tile([P, E], FP32, name="y_t", tag="y_t")
                nc.vector.tensor_add(y_t, ps_f2, x2_t[:, tb, :])
                nc.gpsimd.tensor_add(y_t, y_t, b2_bc)
                nc.sync.dma_start(y_d[ds(c * CT + tb * P, P), :], y_t)

    nc.compile()
    return nc


def make_aux_inputs():
    bf = ml_dtypes.bfloat16
    triu = np.triu(np.ones((P, P), bf))
    mask2 = np.concatenate([np.zeros((P, P), bf), triu], axis=1)
    ident = np.eye(P, dtype=bf)
    ones = np.ones((1, P), bf)
    return {"triu": triu, "mask2": mask2, "ident": ident, "ones": ones}


def weight_inputs(Wq, Wk, Wv, Wo, bo, ln1_g, ln1_b, ln2_g, ln2_b, W1, b1, W2, b2):
    bf = ml_dtypes.bfloat16
    f32 = lambda a: np.ascontiguousarray(np.asarray(a), dtype=np.float32)
    tobf = lambda a: np.ascontiguousarray(np.asarray(a, dtype=np.float32)).astype(bf)
    m = {
        "wq": np.ascontiguousarray(f32(Wq).transpose(1, 0, 2).reshape(E, E)).astype(bf),
        "wk": np.ascontiguousarray(f32(Wk).transpose(1, 0, 2).reshape(E, E)).astype(bf),
        "wv": np.ascontiguousarray(f32(Wv).transpose(1, 0, 2).reshape(E, E)).astype(bf),
        "wo": tobf(Wo),
        "bo": tobf(bo), "g1": f32(ln1_g), "be1": f32(ln1_b),
        "g2": f32(ln2_g), "be2": f32(ln2_b),
        "w1": tobf(W1), "b1": f32(b1), "w2": tobf(W2), "b2": tobf(b2),
    }
    m.update(make_aux_inputs())
    return m

from concourse.bass_utils import run_bass_kernel_spmd

_NC_CACHE = {}


def get_compiled():
    if "nc" not in _NC_CACHE:
        _NC_CACHE["nc"] = build()
    return _NC_CACHE["nc"]


def run_sharded(in_maps, **kwargs):
    nc = get_compiled()
    return run_bass_kernel_spmd(nc, in_maps, core_ids=list(range(NCORES)), **kwargs)


def make_in_maps(x, weights):
    x = np.ascontiguousarray(np.asarray(x), dtype=np.float32)
    in_maps = []
    for c in range(NCORES):
        m = dict(weights)
        m["x"] = np.ascontiguousarray(x[c * BS:(c + 1) * BS].reshape(NTOK, E))
        in_maps.append(m)
    return in_maps


def kernel(x, Wq, Wk, Wv, Wo, bo, ln1_g, ln1_b, ln2_g, ln2_b, W1, b1, W2, b2):
    weights = weight_inputs(Wq, Wk, Wv, Wo, bo, ln1_g, ln1_b,
                            ln2_g, ln2_b, W1, b1, W2, b2)
    res = run_sharded(make_in_maps(x, weights))
    y = np.stack([res.results[c]["y"].reshape(BS, T, E)
                  for c in range(NCORES)], axis=0).reshape(B, T, E)
    return np.ascontiguousarray(y.astype(np.float32))

